# revision 32
# baseline (speedup 1.0000x reference)
"""Single-head causal attention (CustomHead) on 8 Trainium2 NeuronCores.

Reference (per batch b):
    q = x Wq^T ; k = x Wk^T ; v = x Wv^T          (x: [T, C], W*: [H, C])
    S = q k^T * C**-0.5 ; causal mask ; softmax ; out = P v    ([T, H])

Sharding: data-parallel over batch B=32 across 8 cores (4 batches/core).
Each core holds full Wq/Wk/Wv.

Kernel plan per core (T=2048, C=1024, H=128), all bf16 matmuls, fp32 accum:
  - x loaded with SWDGE cast-DMA (fp32->bf16 in the DMA engine).
  - PE-transpose x into x^T (projections contract over C, which must sit
    on the partition dim); DVE copies PSUM->SBUF.
  - q^T/k^T/v^T = W @ x^T with W pre-transposed via one DMA-XBAR
    transpose each at setup.
  - v^T -> natural [s, h] + ones column via four XBAR transposes per
    batch into a strided [128, 16, 144] tile (ones col at 128 makes
    P^T @ [v | 1] accumulate numerator and softmax denominator
    together).
  - Scores computed transposed: S^T[s, t] = kT(s-block) vs qT, 512-wide
    PSUM chunks, one exp (ACT) per chunk; no max-subtraction (scores
    are bounded, exp is safe in fp32).  P^T rows stored bf16 in
    right-sized tiles ([128, 2048-128*ss]).
  - Causal handling: S^T block-row ss only computes t >= 512*(ss//4);
    the diagonal 128x128 block is masked by an upper-triangular 0/1
    multiply after exp; nothing below is ever read.
  - The batch loop is software-pipelined: batch b+1's transpose and
    projection work is emitted in program order between batch b's
    score-row S matmuls and P.V matmuls, so the PE FIFO has useful
    work while the ACT exp chain drains.  Unit order is load-aware:
    projections over the first x^T half run between the two transpose
    half-passes, giving the second half's x tiles time to arrive.
  - Batch 0 uses per-x-tile transposes so the PE starts as soon as the
    first 128-row tile lands instead of waiting for eight.
"""

import numpy as np

B, T, C, H = 32, 2048, 1024, 128
NCORES = 8
BL = B // NCORES  # batches per core

_CACHE = {}


def _build():
    import concourse.bass as bass
    import concourse.tile as tile
    from concourse import bacc, mybir
    from concourse.masks import make_identity, make_upper_triangular

    f32 = mybir.dt.float32
    bf16 = mybir.dt.bfloat16
    Exp = mybir.ActivationFunctionType.Exp
    SC = float(C) ** -0.5

    nc = bacc.Bacc(
        "TRN2",
        target_bir_lowering=False,
        debug=False,
        enable_asserts=False,
        num_devices=NCORES,
    )
    x_ap = nc.dram_tensor("x", [BL, T, C], f32, kind="ExternalInput").ap()
    wk_ap = nc.dram_tensor("Wk", [H, C], f32, kind="ExternalInput").ap()
    wq_ap = nc.dram_tensor("Wq", [H, C], f32, kind="ExternalInput").ap()
    wv_ap = nc.dram_tensor("Wv", [H, C], f32, kind="ExternalInput").ap()
    out_ap = nc.dram_tensor("out", [BL, T, H], f32, kind="ExternalOutput").ap()

    with tile.TileContext(nc) as tc:
        from contextlib import ExitStack

        with ExitStack() as ctx:
            consts = ctx.enter_context(tc.tile_pool(name="consts", bufs=1))
            wstage = ctx.enter_context(tc.tile_pool(name="wstage", bufs=1))
            xbf_p = ctx.enter_context(tc.tile_pool(name="xbf", bufs=16))
            xt_p = ctx.enter_context(tc.tile_pool(name="xt", bufs=9))
            qk_p = ctx.enter_context(tc.tile_pool(name="qk", bufs=2))
            va_p = ctx.enter_context(tc.tile_pool(name="va", bufs=2))
            pr_p = ctx.enter_context(tc.tile_pool(name="prow", bufs=1))
            osb_p = ctx.enter_context(tc.tile_pool(name="osb", bufs=2))
            rc_p = ctx.enter_context(tc.tile_pool(name="rc", bufs=4))
            trans_ps = ctx.enter_context(
                tc.tile_pool(name="trans_ps", bufs=2, space="PSUM")
            )
            mm_ps = ctx.enter_context(tc.tile_pool(name="mm_ps", bufs=2, space="PSUM"))
            srow_ps = ctx.enter_context(
                tc.tile_pool(name="srow_ps", bufs=2, space="PSUM")
            )
            pv_ps = ctx.enter_context(tc.tile_pool(name="pv_ps", bufs=2, space="PSUM"))

            ident = consts.tile([128, 128], bf16)
            make_identity(nc, ident)

            # trimask[s, t] = 1 if s <= t else 0 (valid region of the
            # transposed diagonal block)
            trimask = consts.tile([128, 128], bf16)
            make_upper_triangular(nc, trimask, val=1.0, diag=True)

            def emit_weights():
                # after the batch-0 x loads so the GpSimd/sync queues give
                # the x DMAs a head start
                WT = {}
                for name, wap in (("q", wq_ap), ("k", wk_ap), ("v", wv_ap)):
                    wnat = wstage.tile([128, C], f32, tag="wnat")
                    nc.sync.dma_start(out=wnat, in_=wap)
                    wbf = wstage.tile([128, C], bf16, tag="wbf")
                    nc.vector.tensor_copy(out=wbf, in_=wnat)
                    wt3 = consts.tile(
                        [128, 8, 128], bf16, tag=f"wt{name}", name=f"wt{name}"
                    )
                    nc.sync.dma_start(out=wt3, in_=wbf, transpose=True)
                    WT[name] = wt3
                return WT

            def emit_loads(b):
                xbfs = []
                for tt in range(16):
                    xb = xbf_p.tile([128, C], bf16, tag="xb", name=f"xb{tt}")
                    nc.gpsimd.dma_start(
                        out=xb, in_=x_ap[b, 128 * tt : 128 * (tt + 1), :]
                    )
                    xbfs.append(xb)
                return xbfs

            def make_b_units(b, xbfs, fine_prologue=False):
                """Per-batch transpose/projection work as a list of closures;
                each one is a PE-queue-sized unit.  Order is load-aware."""
                xts = [
                    xt_p.tile([128, T], bf16, name=f"xt{cc}", tag="xt")
                    for cc in range(8)
                ]
                qT = qk_p.tile([128, T], bf16, tag="qT")
                kT = qk_p.tile([128, T], bf16, tag="kT")
                vT = qk_p.tile([128, T], bf16, tag="vT")
                va = va_p.tile([128, 16, 144], bf16)

                def trans_unit(tt8, cc):
                    # one [128, 1024] strip of x^T chunk cc via 8 PE
                    # transposes + 1 DVE copy
                    def f():
                        ps = trans_ps.tile([128, 1024], bf16)
                        for m in range(8):
                            nc.tensor.transpose(
                                ps[:, 128 * m : 128 * (m + 1)],
                                xbfs[8 * tt8 + m][:, 128 * cc : 128 * (cc + 1)],
                                ident,
                            )
                        nc.vector.tensor_copy(
                            out=xts[cc][:, 1024 * tt8 : 1024 * (tt8 + 1)], in_=ps
                        )
                    return f

                def fine_trans_unit(tt):
                    # all 8 chunks of ONE x tile (batch-0 prologue: starts
                    # as soon as each 128-row tile lands)
                    def f():
                        ps = trans_ps.tile([128, 1024], bf16)
                        for cc in range(8):
                            nc.tensor.transpose(
                                ps[:, 128 * cc : 128 * (cc + 1)],
                                xbfs[tt][:, 128 * cc : 128 * (cc + 1)],
                                ident,
                            )
                        for cc in range(8):
                            nc.vector.tensor_copy(
                                out=xts[cc][:, 128 * tt : 128 * (tt + 1)],
                                in_=ps[:, 128 * cc : 128 * (cc + 1)],
                            )
                    return f

                def proj_unit(wt3, dst, s4, with_va=False):
                    def f():
                        ps = mm_ps.tile([128, 512], f32, tag="mm", name="psp")
                        for cc in range(8):
                            nc.tensor.matmul(
                                ps,
                                wt3[:, cc, :],
                                xts[cc][:, 512 * s4 : 512 * (s4 + 1)],
                                start=(cc == 0),
                                stop=(cc == 7),
                            )
                        nc.scalar.copy(out=dst[:, 512 * s4 : 512 * (s4 + 1)], in_=ps)
                        if with_va:
                            if s4 == 0:
                                nc.gpsimd.memset(va[:, :, 128:129], 1.0)
                            nc.sync.dma_start(
                                out=va[:, 4 * s4 : 4 * s4 + 4, 0:128],
                                in_=vT[:, 512 * s4 : 512 * (s4 + 1)],
                                transpose=True,
                            )
                    return f

                units = []
                if fine_prologue:
                    # stage-wise: 4 x tiles + the projection slice they
                    # unlock, so scores column tq can start after stage tq
                    for s4 in range(4):
                        for tt in range(4 * s4, 4 * s4 + 4):
                            units.append(fine_trans_unit(tt))
                        units.append(proj_unit(WT["q"], qT, s4))
                        units.append(proj_unit(WT["k"], kT, s4))
                        units.append(proj_unit(WT["v"], vT, s4, with_va=True))
                else:
                    for cc in range(8):
                        units.append(trans_unit(0, cc))
                    for s4 in (0, 1):
                        units.append(proj_unit(WT["q"], qT, s4))
                        units.append(proj_unit(WT["k"], kT, s4))
                        units.append(proj_unit(WT["v"], vT, s4, with_va=True))
                    for cc in range(8):
                        units.append(trans_unit(1, cc))
                    for s4 in (2, 3):
                        units.append(proj_unit(WT["q"], qT, s4))
                        units.append(proj_unit(WT["k"], kT, s4))
                        units.append(proj_unit(WT["v"], vT, s4, with_va=True))
                state = dict(qT=qT, kT=kT, va=va, tail=[])
                return units, state

            def emit_units(units, n):
                for _ in range(n):
                    if units:
                        units.pop(0)()

            def emit_scores(b, st, next_units, own_units=None):
                # Column-major: score column tq only needs projection
                # slices s4 <= tq, so batch 0 starts its scores while its
                # own x is still streaming in (own_units drained per
                # column), and later batches fill PE stalls with next
                # batch's work (next_units, cols 2-3).
                qT, kT, va = st["qT"], st["kT"], st["va"]
                out_sb = osb_p.tile([128, 16 * H], f32)
                prows = []
                for ss in range(16):
                    prows.append(
                        pr_p.tile(
                            [128, T - 128 * ss],
                            bf16,
                            tag=f"pr{ss}",
                            name=f"pr{ss}",
                            bufs=2 if ss < 4 else 1,
                        )
                    )
                tail = st.get("tail") or []
                for tq in range(4):
                    if own_units is not None and tq >= 1:
                        # batch 0: this column's own x tiles + projection
                        # slice must precede the column (and must precede
                        # any next-batch fills, whose loads wait on these
                        # tiles' buffer slots)
                        emit_units(own_units, 7)
                    c0 = 512 * tq
                    for ss in range(4 * tq + 4):
                        pb = 128 * ss
                        x0 = max(pb, c0)  # first causal-needed column
                        d0 = x0 - c0
                        sh = srow_ps.tile([128, 512], f32)
                        nc.tensor.matmul(
                            sh[:, d0:512],
                            kT[:, pb : pb + 128],
                            qT[:, x0 : c0 + 512],
                            start=True,
                            stop=True,
                        )
                        nc.scalar.activation(
                            out=prows[ss][:, x0 - pb : c0 + 512 - pb],
                            in_=sh[:, d0:512],
                            func=Exp,
                            scale=SC,
                        )
                        if (
                            own_units is None
                            and next_units
                            and (tq == 3 or (tq == 2 and ss >= 6))
                        ):
                            emit_units(next_units, 1)
                    for a in range(4):
                        ss = 4 * tq + a
                        pb = 128 * ss
                        pr = prows[ss]
                        nc.vector.tensor_mul(pr[:, 0:128], pr[:, 0:128], trimask)
                        if tq < 2 and tail:
                            # this batch's held-back s4=3 projections fill
                            # the early columns (next batch's x is still
                            # loading)
                            emit_units(tail, 1)
                        elif own_units is None and next_units and tq >= 2:
                            emit_units(next_units, 1)
                        pv = pv_ps.tile([128, H + 1], f32)
                        for j in range(ss + 1):
                            nc.tensor.matmul(
                                pv,
                                prows[j][:, pb - 128 * j : pb - 128 * j + 128],
                                va[:, j, 0 : H + 1],
                                start=(j == 0),
                                stop=(j == ss),
                            )
                        rc = rc_p.tile([128, 1], f32)
                        nc.vector.reciprocal(rc, pv[:, 128:129])
                        nc.vector.tensor_mul(
                            out_sb[:, H * ss : H * (ss + 1)],
                            pv[:, 0:128],
                            rc.broadcast_to([128, H]),
                        )
                if own_units is not None:
                    emit_units(own_units, len(own_units))
                if tail:
                    emit_units(tail, len(tail))
                if next_units:
                    # keep the last 3 units (s4=3 projections) as the next
                    # score phase's early-column fill
                    emit_units(next_units, len(next_units) - 3)
                # out_sb[p, (g h)] -> out[b, 128g+p, h]; split DMAs so the
                # final transfer after the last normalize is small
                np_split = 4 if b == BL - 1 else 2
                npc = 2048 // np_split
                for hh in range(np_split):
                    nc.sync.dma_start(
                        out=out_ap[b, npc * hh : npc * (hh + 1), :].rearrange(
                            "(g p) h -> p g h", p=128
                        ),
                        in_=out_sb[
                            :, npc // 128 * H * hh : npc // 128 * H * (hh + 1)
                        ].rearrange("p (g h) -> p g h", h=H),
                    )

            # --- software-pipelined batch loop ---
            xbfs = emit_loads(0)
            WT = emit_weights()
            units, st = make_b_units(0, xbfs, fine_prologue=True)
            # prologue: first four x tiles transposed + s4=0 projections,
            # then start scores column 0 while the rest streams
            emit_units(units, 7)
            for b in range(BL):
                if b + 1 < BL:
                    xbfs = emit_loads(b + 1)
                    next_units, next_st = make_b_units(b + 1, xbfs)
                else:
                    next_units, next_st = [], None
                emit_scores(
                    b, st, next_units, own_units=units if b == 0 else None
                )
                if next_st is not None:
                    next_st["tail"] = next_units  # leftover s4=3 units
                st = next_st

    nc.compile()
    return nc


def _get_nc():
    if "nc" not in _CACHE:
        _CACHE["nc"] = _build()
    return _CACHE["nc"]


def kernel(x, Wk, Wq, Wv, _trace=False):
    from concourse.bass_utils import run_bass_kernel_spmd

    x = np.ascontiguousarray(np.asarray(x, dtype=np.float32))
    Wk = np.ascontiguousarray(np.asarray(Wk, dtype=np.float32))
    Wq = np.ascontiguousarray(np.asarray(Wq, dtype=np.float32))
    Wv = np.ascontiguousarray(np.asarray(Wv, dtype=np.float32))
    assert x.shape == (B, T, C)

    nc = _get_nc()
    in_maps = [
        {"x": x[i * BL : (i + 1) * BL], "Wk": Wk, "Wq": Wq, "Wv": Wv}
        for i in range(NCORES)
    ]
    res = run_bass_kernel_spmd(nc, in_maps, list(range(NCORES)), trace=_trace)
    out = np.concatenate([res.results[i]["out"] for i in range(NCORES)], axis=0)
    if _trace:
        _CACHE["last_results"] = res
    return out


# revision 33
# speedup vs baseline: 1.0162x; 1.0162x over previous
"""Single-head causal attention (CustomHead) on 8 Trainium2 NeuronCores.

Reference (per batch b):
    q = x Wq^T ; k = x Wk^T ; v = x Wv^T          (x: [T, C], W*: [H, C])
    S = q k^T * C**-0.5 ; causal mask ; softmax ; out = P v    ([T, H])

Sharding: data-parallel over batch B=32 across 8 cores (4 batches/core).
Each core holds full Wq/Wk/Wv.

Kernel plan per core (T=2048, C=1024, H=128), all bf16 matmuls, fp32 accum:
  - x is loaded with SWDGE cast-DMA (fp32->bf16 in the DMA engine), so no
    vector-engine cast pass is needed.
  - PE-transpose x into x^T (every projection contracts over C, which must
    sit on the partition dim); DVE copies PSUM->SBUF.
  - q^T/k^T/v^T = W @ x^T; v^T is PE-transposed back to natural [s, h].
  - Scores computed transposed: S^T[s, t] = kT(s-block) vs qT, 1024-wide
    PSUM chunks, one exp (ACT) per chunk.  No max-subtraction (scores are
    bounded: |S * C^-0.5| < ~1, exp is safe in fp32); the row-sum comes for
    free from a ones-column appended to v (P^T @ [v | 1] accumulates both
    numerator and denominator).
  - Causal handling: S^T block-row ss only computes t >= 512*(ss//4); the
    diagonal 128x128 block is masked by an upper-triangular 0/1 multiply
    after exp; everything below is never read.
  - Output rows are normalized into one SBUF tile per batch and written
    with a single 1 MB DMA.
"""

import numpy as np

B, T, C, H = 32, 2048, 1024, 128
NCORES = 8
BL = B // NCORES  # batches per core

_CACHE = {}


def _build():
    import concourse.bass as bass
    import concourse.tile as tile
    from concourse import bacc, mybir
    from concourse.masks import make_identity, make_upper_triangular

    f32 = mybir.dt.float32
    bf16 = mybir.dt.bfloat16
    Exp = mybir.ActivationFunctionType.Exp
    SC = float(C) ** -0.5  # 1/32 exactly

    nc = bacc.Bacc(
        "TRN2",
        target_bir_lowering=False,
        debug=False,
        enable_asserts=False,
        num_devices=NCORES,
    )
    x_ap = nc.dram_tensor("x", [BL, T, C], f32, kind="ExternalInput").ap()
    wk_ap = nc.dram_tensor("Wk", [H, C], f32, kind="ExternalInput").ap()
    wq_ap = nc.dram_tensor("Wq", [H, C], f32, kind="ExternalInput").ap()
    wv_ap = nc.dram_tensor("Wv", [H, C], f32, kind="ExternalInput").ap()
    out_ap = nc.dram_tensor("out", [BL, T, H], f32, kind="ExternalOutput").ap()

    with tile.TileContext(nc) as tc:
        from contextlib import ExitStack

        with ExitStack() as ctx:
            consts = ctx.enter_context(tc.tile_pool(name="consts", bufs=1))
            wstage = ctx.enter_context(tc.tile_pool(name="wstage", bufs=2))
            xbf_p = ctx.enter_context(tc.tile_pool(name="xbf", bufs=18))
            xt_p = ctx.enter_context(tc.tile_pool(name="xt", bufs=9))
            qk_p = ctx.enter_context(tc.tile_pool(name="qk", bufs=2))
            va_p = ctx.enter_context(tc.tile_pool(name="va", bufs=2))
            pr_p = ctx.enter_context(tc.tile_pool(name="prow", bufs=17))
            osb_p = ctx.enter_context(tc.tile_pool(name="osb", bufs=2))
            rc_p = ctx.enter_context(tc.tile_pool(name="rc", bufs=4))
            trans_ps = ctx.enter_context(
                tc.tile_pool(name="trans_ps", bufs=2, space="PSUM")
            )
            mm_ps = ctx.enter_context(tc.tile_pool(name="mm_ps", bufs=2, space="PSUM"))
            srow_ps = ctx.enter_context(
                tc.tile_pool(name="srow_ps", bufs=2, space="PSUM")
            )
            pv_ps = ctx.enter_context(tc.tile_pool(name="pv_ps", bufs=2, space="PSUM"))

            ident = consts.tile([128, 128], bf16)
            make_identity(nc, ident)

            # trimask[s, t] = 1 if s <= t else 0 (valid region of the
            # transposed diagonal block)
            trimask = consts.tile([128, 128], bf16)
            make_upper_triangular(nc, trimask, val=1.0, diag=True)

            # --- weights: load, cast, XBAR-transpose to [c%128, cc, h] ---
            WT = {}
            for name, wap in (("q", wq_ap), ("k", wk_ap), ("v", wv_ap)):
                wnat = wstage.tile([128, C], f32, tag="wnat")
                nc.sync.dma_start(out=wnat, in_=wap)
                wbf = wstage.tile([128, C], bf16, tag="wbf")
                nc.vector.tensor_copy(out=wbf, in_=wnat)
                wt = consts.tile([128, 8, 128], bf16, tag=f"wt_{name}")
                nc.sync.dma_start(out=wt, in_=wbf, transpose=True)
                WT[name] = wt

            for b in range(BL):
                # --- x load: SWDGE cast-DMA fp32 -> bf16, natural layout ---
                xbfs = []
                for tt in range(16):
                    xb = xbf_p.tile([128, C], bf16)
                    nc.gpsimd.dma_start(
                        out=xb, in_=x_ap[b, 128 * tt : 128 * (tt + 1), :]
                    )
                    xbfs.append(xb)

                # --- x -> x^T (bf16) via PE transpose ---
                xts = [
                    xt_p.tile([128, T], bf16, name=f"xt{cc}", tag="xt")
                    for cc in range(8)
                ]
                for tt8 in range(2):
                    for cc in range(8):
                        ps = trans_ps.tile([128, 1024], bf16)
                        for m in range(8):
                            nc.tensor.transpose(
                                ps[:, 128 * m : 128 * (m + 1)],
                                xbfs[8 * tt8 + m][:, 128 * cc : 128 * (cc + 1)],
                                ident,
                            )
                        nc.vector.tensor_copy(
                            out=xts[cc][:, 1024 * tt8 : 1024 * (tt8 + 1)], in_=ps
                        )

                # --- projections ---
                qT = qk_p.tile([128, T], bf16)
                kT = qk_p.tile([128, T], bf16)
                for wt, dst in ((WT["q"], qT), (WT["k"], kT)):
                    for tt4 in range(4):
                        ps = mm_ps.tile([128, 512], f32)
                        for cc in range(8):
                            nc.tensor.matmul(
                                ps,
                                wt[:, cc, :],
                                xts[cc][:, 512 * tt4 : 512 * (tt4 + 1)],
                                start=(cc == 0),
                                stop=(cc == 7),
                            )
                        nc.scalar.copy(
                            out=dst[:, 512 * tt4 : 512 * (tt4 + 1)], in_=ps
                        )
                # v^T = Wv @ x^T (like q/k), then PE-transpose back to natural
                # [s, h] blocks for the P.V contraction.
                vT = qk_p.tile([128, T], bf16)
                va = va_p.tile([128, 16, 144], bf16)
                for tt4 in range(4):
                    ps = mm_ps.tile([128, 512], f32)
                    for cc in range(8):
                        nc.tensor.matmul(
                            ps,
                            WT["v"][:, cc, :],
                            xts[cc][:, 512 * tt4 : 512 * (tt4 + 1)],
                            start=(cc == 0),
                            stop=(cc == 7),
                        )
                    nc.scalar.copy(out=vT[:, 512 * tt4 : 512 * (tt4 + 1)], in_=ps)
                    if tt4 == 0:
                        nc.gpsimd.memset(va[:, :, 128:129], 1.0)
                    nc.sync.dma_start(
                        out=va[:, 4 * tt4 : 4 * tt4 + 4, 0:128],
                        in_=vT[:, 512 * tt4 : 512 * (tt4 + 1)],
                        transpose=True,
                    )

                # --- scores (transposed), exp, and P.V interleaved ---
                out_sb = osb_p.tile([128, 16 * H], f32)
                prows = []
                for ss in range(16):
                    pr = pr_p.tile([128, T], bf16)
                    prows.append(pr)
                    for tq in range(ss // 4, 4):
                        c0 = 512 * tq
                        x0 = max(128 * ss, c0)  # first causal-needed column
                        d0 = x0 - c0
                        sh = srow_ps.tile([128, 512], f32)
                        nc.tensor.matmul(
                            sh[:, d0:512],
                            kT[:, 128 * ss : 128 * (ss + 1)],
                            qT[:, x0 : c0 + 512],
                            start=True,
                            stop=True,
                        )
                        nc.scalar.activation(
                            out=pr[:, x0 : c0 + 512],
                            in_=sh[:, d0:512],
                            func=Exp,
                            scale=SC,
                        )
                    nc.vector.tensor_mul(
                        pr[:, 128 * ss : 128 * (ss + 1)],
                        pr[:, 128 * ss : 128 * (ss + 1)],
                        trimask,
                    )
                    pv = pv_ps.tile([128, H + 1], f32)
                    for j in range(ss + 1):
                        nc.tensor.matmul(
                            pv,
                            prows[j][:, 128 * ss : 128 * (ss + 1)],
                            va[:, j, 0 : H + 1],
                            start=(j == 0),
                            stop=(j == ss),
                        )
                    rc = rc_p.tile([128, 1], f32)
                    nc.vector.reciprocal(rc, pv[:, 128:129])
                    nc.vector.tensor_mul(
                        out_sb[:, H * ss : H * (ss + 1)],
                        pv[:, 0:128],
                        rc.broadcast_to([128, H]),
                    )
                # out_sb[p, (g h)] -> out[b, 128g+p, h]; split DMAs so the
                # final transfer after the last normalize is small
                np_split = 4 if b == BL - 1 else 2
                npc = 2048 // np_split
                for hh in range(np_split):
                    nc.sync.dma_start(
                        out=out_ap[b, npc * hh : npc * (hh + 1), :].rearrange(
                            "(g p) h -> p g h", p=128
                        ),
                        in_=out_sb[
                            :, npc // 128 * H * hh : npc // 128 * H * (hh + 1)
                        ].rearrange("p (g h) -> p g h", h=H),
                    )

    nc.compile()
    return nc


def _get_nc():
    if "nc" not in _CACHE:
        _CACHE["nc"] = _build()
    return _CACHE["nc"]


def kernel(x, Wk, Wq, Wv, _trace=False):
    from concourse.bass_utils import run_bass_kernel_spmd

    x = np.ascontiguousarray(np.asarray(x, dtype=np.float32))
    Wk = np.ascontiguousarray(np.asarray(Wk, dtype=np.float32))
    Wq = np.ascontiguousarray(np.asarray(Wq, dtype=np.float32))
    Wv = np.ascontiguousarray(np.asarray(Wv, dtype=np.float32))
    assert x.shape == (B, T, C)

    nc = _get_nc()
    in_maps = [
        {"x": x[i * BL : (i + 1) * BL], "Wk": Wk, "Wq": Wq, "Wv": Wv}
        for i in range(NCORES)
    ]
    res = run_bass_kernel_spmd(nc, in_maps, list(range(NCORES)), trace=_trace)
    out = np.concatenate([res.results[i]["out"] for i in range(NCORES)], axis=0)
    if _trace:
        _CACHE["last_results"] = res
    return out



# revision 35
# speedup vs baseline: 1.0512x; 1.0344x over previous
"""Single-head causal attention (CustomHead) on 8 Trainium2 NeuronCores.

Reference (per batch b):
    q = x Wq^T ; k = x Wk^T ; v = x Wv^T          (x: [T, C], W*: [H, C])
    S = q k^T * C**-0.5 ; causal mask ; softmax ; out = P v    ([T, H])

Sharding: data-parallel over batch B=32 across 8 cores (4 batches/core).
Each core holds full Wq/Wk/Wv.

Kernel plan per core (T=2048, C=1024, H=128), all bf16 matmuls, fp32 accum:
  - x loaded with SWDGE cast-DMA (fp32->bf16 in the DMA engine),
    dispatched before anything else so HBM streams immediately.
  - PE-transpose x into x^T (projections contract over C, which must sit
    on the partition dim); DVE copies PSUM->SBUF.
  - q^T/k^T/v^T = W @ x^T; v^T PE-transposed back to natural [s, h]
    blocks with a ones column (P^T @ [v | 1] accumulates numerator and
    softmax denominator together).
  - Scores computed transposed: S^T[s, t] = kT(s-block) vs qT, 512-wide
    PSUM chunks, one exp (ACT) per chunk; no max-subtraction (scores
    are bounded, exp is safe in fp32).  P^T rows stored bf16 in
    right-sized tiles ([128, 2048-128*ss]).
  - Causal handling: S^T block-row ss only computes t >= 512*(ss//4);
    the diagonal 128x128 block is masked by an upper-triangular 0/1
    multiply after exp; nothing below is ever read.
  - The batch loop is software-pipelined: batch b+1's transpose and
    projection work is emitted in program order between batch b's
    score-row S matmuls and P.V matmuls, so the PE FIFO has useful
    work while the ACT exp chain drains.  Unit order is load-aware:
    projections over the first x^T half run between the two transpose
    half-passes, giving the second half's x tiles time to arrive.
  - Batch 0 uses per-x-tile transposes so the PE starts as soon as the
    first 128-row tile lands instead of waiting for eight.
"""

import numpy as np

B, T, C, H = 32, 2048, 1024, 128
NCORES = 8
BL = B // NCORES  # batches per core

_CACHE = {}


def _build():
    import concourse.bass as bass
    import concourse.tile as tile
    from concourse import bacc, mybir
    from concourse.masks import make_identity, make_upper_triangular

    f32 = mybir.dt.float32
    bf16 = mybir.dt.bfloat16
    Exp = mybir.ActivationFunctionType.Exp
    SC = float(C) ** -0.5

    nc = bacc.Bacc(
        "TRN2",
        target_bir_lowering=False,
        debug=False,
        enable_asserts=False,
        num_devices=NCORES,
    )
    x_ap = nc.dram_tensor("x", [BL, T, C], f32, kind="ExternalInput").ap()
    wk_ap = nc.dram_tensor("Wk", [H, C], f32, kind="ExternalInput").ap()
    wq_ap = nc.dram_tensor("Wq", [H, C], f32, kind="ExternalInput").ap()
    wv_ap = nc.dram_tensor("Wv", [H, C], f32, kind="ExternalInput").ap()
    out_ap = nc.dram_tensor("out", [BL, T, H], f32, kind="ExternalOutput").ap()

    with tile.TileContext(nc) as tc:
        from contextlib import ExitStack

        with ExitStack() as ctx:
            consts = ctx.enter_context(tc.tile_pool(name="consts", bufs=1))
            wstage = ctx.enter_context(tc.tile_pool(name="wstage", bufs=1))
            xbf_p = ctx.enter_context(tc.tile_pool(name="xbf", bufs=16))
            xt_p = ctx.enter_context(tc.tile_pool(name="xt", bufs=9))
            qk_p = ctx.enter_context(tc.tile_pool(name="qk", bufs=2))
            va_p = ctx.enter_context(tc.tile_pool(name="va", bufs=20))
            pr_p = ctx.enter_context(tc.tile_pool(name="prow", bufs=1))
            osb_p = ctx.enter_context(tc.tile_pool(name="osb", bufs=2))
            rc_p = ctx.enter_context(tc.tile_pool(name="rc", bufs=4))
            trans_ps = ctx.enter_context(
                tc.tile_pool(name="trans_ps", bufs=2, space="PSUM")
            )
            mm_ps = ctx.enter_context(tc.tile_pool(name="mm_ps", bufs=2, space="PSUM"))
            srow_ps = ctx.enter_context(
                tc.tile_pool(name="srow_ps", bufs=2, space="PSUM")
            )
            pv_ps = ctx.enter_context(tc.tile_pool(name="pv_ps", bufs=2, space="PSUM"))

            def emit_loads(b):
                xbfs = []
                for tt in range(16):
                    xb = xbf_p.tile([128, C], bf16, tag="xb", name=f"xb{tt}")
                    nc.gpsimd.dma_start(
                        out=xb, in_=x_ap[b, 128 * tt : 128 * (tt + 1), :]
                    )
                    xbfs.append(xb)
                return xbfs

            # batch-0 x loads dispatched first so HBM streams immediately
            xbfs0 = emit_loads(0)

            ident = consts.tile([128, 128], bf16)
            make_identity(nc, ident)

            # trimask[s, t] = 1 if s <= t else 0 (valid region of the
            # transposed diagonal block)
            trimask = consts.tile([128, 128], bf16)
            make_upper_triangular(nc, trimask, val=1.0, diag=True)

            # --- weights: load, cast, PE-transpose into W^T [c, h] chunks ---
            WT = {}
            for name, wap in (("q", wq_ap), ("k", wk_ap), ("v", wv_ap)):
                wnat = wstage.tile([128, C], f32, tag="wnat")
                nc.sync.dma_start(out=wnat, in_=wap)
                wbf = wstage.tile([128, C], bf16, tag="wbf")
                nc.vector.tensor_copy(out=wbf, in_=wnat)
                wt = consts.tile([128, C], bf16, tag=f"wt_{name}", name=f"wt{name}")
                for g in range(2):
                    ps = trans_ps.tile([128, 512], bf16, tag="ps")
                    for m in range(4):
                        cc = 4 * g + m
                        nc.tensor.transpose(
                            ps[:, 128 * m : 128 * (m + 1)],
                            wbf[:, 128 * cc : 128 * (cc + 1)],
                            ident,
                        )
                    nc.vector.tensor_copy(out=wt[:, 512 * g : 512 * (g + 1)], in_=ps)
                WT[name] = wt

            def make_b_units(b, xbfs, fine_prologue=False):
                """Per-batch transpose/projection work as a list of closures;
                each one is a PE-queue-sized unit.  Order is load-aware."""
                xts = [
                    xt_p.tile([128, T], bf16, name=f"xt{cc}", tag="xt")
                    for cc in range(8)
                ]
                qT = qk_p.tile([128, T], bf16, tag="qT")
                kT = qk_p.tile([128, T], bf16, tag="kT")
                vT = qk_p.tile([128, T], bf16, tag="vT")
                vas = []

                def trans_unit(tt8, cc):
                    def f():
                        ps = trans_ps.tile([128, 1024], bf16)
                        for m in range(8):
                            nc.tensor.transpose(
                                ps[:, 128 * m : 128 * (m + 1)],
                                xbfs[8 * tt8 + m][:, 128 * cc : 128 * (cc + 1)],
                                ident,
                            )
                        nc.vector.tensor_copy(
                            out=xts[cc][:, 1024 * tt8 : 1024 * (tt8 + 1)], in_=ps
                        )
                    return f

                def fine_trans_unit(tt):
                    # all 8 chunks of ONE x tile (batch-0 prologue)
                    def f():
                        ps = trans_ps.tile([128, 1024], bf16)
                        for cc in range(8):
                            nc.tensor.transpose(
                                ps[:, 128 * cc : 128 * (cc + 1)],
                                xbfs[tt][:, 128 * cc : 128 * (cc + 1)],
                                ident,
                            )
                        for cc in range(8):
                            nc.vector.tensor_copy(
                                out=xts[cc][:, 128 * tt : 128 * (tt + 1)],
                                in_=ps[:, 128 * cc : 128 * (cc + 1)],
                            )
                    return f

                def proj_unit(wt, dst, s4):
                    def f():
                        ps = mm_ps.tile([128, 512], f32, tag="mm", name="psp")
                        for cc in range(8):
                            nc.tensor.matmul(
                                ps,
                                wt[:, 128 * cc : 128 * (cc + 1)],
                                xts[cc][:, 512 * s4 : 512 * (s4 + 1)],
                                start=(cc == 0),
                                stop=(cc == 7),
                            )
                        nc.scalar.copy(out=dst[:, 512 * s4 : 512 * (s4 + 1)], in_=ps)
                    return f

                def vas_unit(s4):
                    # v^T slice -> 4 natural [s, h] blocks via PE transpose
                    def f():
                        for a in range(4):
                            ss = 4 * s4 + a
                            psv = trans_ps.tile([128, 512], bf16, tag="ps")
                            nc.tensor.transpose(
                                psv[:, 0:128],
                                vT[:, 128 * ss : 128 * (ss + 1)],
                                ident,
                            )
                            va = va_p.tile([128, H + 1], bf16, tag="va", name="va")
                            nc.vector.tensor_copy(out=va[:, 0:128], in_=psv[:, 0:128])
                            nc.gpsimd.memset(va[:, 128:129], 1.0)
                            vas.append(va)
                    return f

                units = []
                if fine_prologue:
                    for tt in range(16):
                        units.append(fine_trans_unit(tt))
                    for s4 in range(4):
                        units.append(proj_unit(WT["q"], qT, s4))
                        units.append(proj_unit(WT["k"], kT, s4))
                        units.append(proj_unit(WT["v"], vT, s4))
                        units.append(vas_unit(s4))
                else:
                    for cc in range(8):
                        units.append(trans_unit(0, cc))
                    for s4 in (0, 1):
                        units.append(proj_unit(WT["q"], qT, s4))
                        units.append(proj_unit(WT["k"], kT, s4))
                        units.append(proj_unit(WT["v"], vT, s4))
                        units.append(vas_unit(s4))
                    for cc in range(8):
                        units.append(trans_unit(1, cc))
                    for s4 in (2, 3):
                        units.append(proj_unit(WT["q"], qT, s4))
                        units.append(proj_unit(WT["k"], kT, s4))
                        units.append(proj_unit(WT["v"], vT, s4))
                        units.append(vas_unit(s4))
                state = dict(qT=qT, kT=kT, vas=vas)
                return units, state

            def emit_units(units, n):
                for _ in range(n):
                    if units:
                        units.pop(0)()

            def emit_scores(b, st, next_units):
                qT, kT, vas = st["qT"], st["kT"], st["vas"]
                out_sb = osb_p.tile([128, 16 * H], f32)
                prows = []
                for ss in range(16):
                    pb = 128 * ss
                    pr = pr_p.tile(
                        [128, T - pb],
                        bf16,
                        tag=f"pr{ss}",
                        name=f"pr{ss}",
                        bufs=2 if ss < 4 else 1,
                    )
                    prows.append(pr)
                    for tq in range(ss // 4, 4):
                        c0 = 512 * tq
                        x0 = max(pb, c0)  # first causal-needed column
                        d0 = x0 - c0
                        sh = srow_ps.tile([128, 512], f32)
                        nc.tensor.matmul(
                            sh[:, d0:512],
                            kT[:, pb : pb + 128],
                            qT[:, x0 : c0 + 512],
                            start=True,
                            stop=True,
                        )
                        nc.scalar.activation(
                            out=pr[:, x0 - pb : c0 + 512 - pb],
                            in_=sh[:, d0:512],
                            func=Exp,
                            scale=SC,
                        )
                    # trimask only needs the first (diagonal) exp chunk;
                    # emit before the fill units so the DVE queue cannot
                    # delay P.V's final matmul
                    nc.vector.tensor_mul(pr[:, 0:128], pr[:, 0:128], trimask)
                    # fill the PE queue with next-batch work while the ACT
                    # exp chain for this row drains (skip early rows: the
                    # next batch's x tiles are still loading)
                    if ss >= 4:
                        emit_units(next_units, 3)
                    pv = pv_ps.tile([128, H + 1], f32)
                    for j in range(ss + 1):
                        nc.tensor.matmul(
                            pv,
                            prows[j][:, pb - 128 * j : pb - 128 * j + 128],
                            vas[j],
                            start=(j == 0),
                            stop=(j == ss),
                        )
                    rc = rc_p.tile([128, 1], f32)
                    nc.vector.reciprocal(rc, pv[:, 128:129])
                    nc.vector.tensor_mul(
                        out_sb[:, H * ss : H * (ss + 1)],
                        pv[:, 0:128],
                        rc.broadcast_to([128, H]),
                    )
                emit_units(next_units, len(next_units))
                # out_sb[p, (g h)] -> out[b, 128g+p, h]; split DMAs so the
                # final transfer after the last normalize is small
                np_split = 4 if b == BL - 1 else 2
                npc = 2048 // np_split
                for hh in range(np_split):
                    nc.sync.dma_start(
                        out=out_ap[b, npc * hh : npc * (hh + 1), :].rearrange(
                            "(g p) h -> p g h", p=128
                        ),
                        in_=out_sb[
                            :, npc // 128 * H * hh : npc // 128 * H * (hh + 1)
                        ].rearrange("p (g h) -> p g h", h=H),
                    )

            # --- software-pipelined batch loop ---
            units, st = make_b_units(0, xbfs0, fine_prologue=True)
            emit_units(units, len(units))  # prologue: batch 0 B-phase flat
            for b in range(BL):
                if b + 1 < BL:
                    xbfs = emit_loads(b + 1)
                    next_units, next_st = make_b_units(b + 1, xbfs)
                else:
                    next_units, next_st = [], None
                emit_scores(b, st, next_units)
                st = next_st

    nc.compile()
    return nc


def _get_nc():
    if "nc" not in _CACHE:
        _CACHE["nc"] = _build()
    return _CACHE["nc"]


def kernel(x, Wk, Wq, Wv, _trace=False):
    from concourse.bass_utils import run_bass_kernel_spmd

    x = np.ascontiguousarray(np.asarray(x, dtype=np.float32))
    Wk = np.ascontiguousarray(np.asarray(Wk, dtype=np.float32))
    Wq = np.ascontiguousarray(np.asarray(Wq, dtype=np.float32))
    Wv = np.ascontiguousarray(np.asarray(Wv, dtype=np.float32))
    assert x.shape == (B, T, C)

    nc = _get_nc()
    in_maps = [
        {"x": x[i * BL : (i + 1) * BL], "Wk": Wk, "Wq": Wq, "Wv": Wv}
        for i in range(NCORES)
    ]
    res = run_bass_kernel_spmd(nc, in_maps, list(range(NCORES)), trace=_trace)
    out = np.concatenate([res.results[i]["out"] for i in range(NCORES)], axis=0)
    if _trace:
        _CACHE["last_results"] = res
    return out


# revision 38
# speedup vs baseline: 1.0828x; 1.0300x over previous
"""Single-head causal attention (CustomHead) on 8 Trainium2 NeuronCores.

Reference (per batch b):
    q = x Wq^T ; k = x Wk^T ; v = x Wv^T          (x: [T, C], W*: [H, C])
    S = q k^T * C**-0.5 ; causal mask ; softmax ; out = P v    ([T, H])

Sharding: data-parallel over batch B=32 across 8 cores (4 batches/core).
Each core holds full Wq/Wk/Wv.

Kernel plan per core (T=2048, C=1024, H=128), all bf16 matmuls, fp32 accum:
  - x is loaded with SWDGE cast-DMA (fp32->bf16 in the DMA engine), so no
    vector-engine cast pass is needed.
  - PE-transpose x into x^T (every projection contracts over C, which must
    sit on the partition dim); DVE copies PSUM->SBUF.
  - q^T/k^T/v^T = W @ x^T; v^T is PE-transposed back to natural [s, h].
  - Scores computed transposed: S^T[s, t] = kT(s-block) vs qT, 1024-wide
    PSUM chunks, one exp (ACT) per chunk.  No max-subtraction (scores are
    bounded: |S * C^-0.5| < ~1, exp is safe in fp32); the row-sum comes for
    free from a ones-column appended to v (P^T @ [v | 1] accumulates both
    numerator and denominator).
  - Causal handling: S^T block-row ss only computes t >= 512*(ss//4); the
    diagonal 128x128 block is masked by an upper-triangular 0/1 multiply
    after exp; everything below is never read.
  - Output rows are normalized into one SBUF tile per batch and written
    with a single 1 MB DMA.
"""

import numpy as np

B, T, C, H = 32, 2048, 1024, 128
NCORES = 8
BL = B // NCORES  # batches per core

_CACHE = {}


def _build():
    import concourse.bass as bass
    import concourse.tile as tile
    from concourse import bacc, mybir
    from concourse.masks import make_identity, make_upper_triangular

    f32 = mybir.dt.float32
    bf16 = mybir.dt.bfloat16
    Exp = mybir.ActivationFunctionType.Exp
    SC = float(C) ** -0.5  # 1/32 exactly

    nc = bacc.Bacc(
        "TRN2",
        target_bir_lowering=False,
        debug=False,
        enable_asserts=False,
        num_devices=NCORES,
    )
    x_ap = nc.dram_tensor("x", [BL, T, C], f32, kind="ExternalInput").ap()
    wk_ap = nc.dram_tensor("Wk", [H, C], f32, kind="ExternalInput").ap()
    wq_ap = nc.dram_tensor("Wq", [H, C], f32, kind="ExternalInput").ap()
    wv_ap = nc.dram_tensor("Wv", [H, C], f32, kind="ExternalInput").ap()
    out_ap = nc.dram_tensor("out", [BL, T, H], f32, kind="ExternalOutput").ap()

    with tile.TileContext(nc) as tc:
        from contextlib import ExitStack

        with ExitStack() as ctx:
            consts = ctx.enter_context(tc.tile_pool(name="consts", bufs=1))
            wstage = ctx.enter_context(tc.tile_pool(name="wstage", bufs=2))
            xbf_p = ctx.enter_context(tc.tile_pool(name="xbf", bufs=18))
            xt_p = ctx.enter_context(tc.tile_pool(name="xt", bufs=9))
            qk_p = ctx.enter_context(tc.tile_pool(name="qk", bufs=2))
            va_p = ctx.enter_context(tc.tile_pool(name="va", bufs=20))
            pr_p = ctx.enter_context(tc.tile_pool(name="prow", bufs=17))
            osb_p = ctx.enter_context(tc.tile_pool(name="osb", bufs=2))
            rc_p = ctx.enter_context(tc.tile_pool(name="rc", bufs=4))
            trans_ps = ctx.enter_context(
                tc.tile_pool(name="trans_ps", bufs=2, space="PSUM")
            )
            mm_ps = ctx.enter_context(tc.tile_pool(name="mm_ps", bufs=2, space="PSUM"))
            srow_ps = ctx.enter_context(
                tc.tile_pool(name="srow_ps", bufs=2, space="PSUM")
            )
            pv_ps = ctx.enter_context(tc.tile_pool(name="pv_ps", bufs=2, space="PSUM"))

            ident = consts.tile([128, 128], bf16)
            make_identity(nc, ident)

            # trimask[s, t] = 1 if s <= t else 0 (valid region of the
            # transposed diagonal block)
            trimask = consts.tile([128, 128], bf16)
            make_upper_triangular(nc, trimask, val=1.0, diag=True)

            # --- weights: load, cast, transpose into W^T [c, h] chunks ---
            WT = {}
            for name, wap in (("q", wq_ap), ("k", wk_ap), ("v", wv_ap)):
                wnat = wstage.tile([128, C], f32, tag="wnat")
                nc.sync.dma_start(out=wnat, in_=wap)
                wbf = wstage.tile([128, C], bf16, tag="wbf")
                nc.vector.tensor_copy(out=wbf, in_=wnat)
                wt = consts.tile([128, C], bf16, tag=f"wt_{name}")
                for g in range(2):
                    ps = trans_ps.tile([128, 512], bf16)
                    for m in range(4):
                        cc = 4 * g + m
                        nc.tensor.transpose(
                            ps[:, 128 * m : 128 * (m + 1)],
                            wbf[:, 128 * cc : 128 * (cc + 1)],
                            ident,
                        )
                    nc.vector.tensor_copy(out=wt[:, 512 * g : 512 * (g + 1)], in_=ps)
                WT[name] = wt

            for b in range(BL):
                # --- x load: SWDGE cast-DMA fp32 -> bf16, natural layout ---
                xbfs = []
                for tt in range(16):
                    xb = xbf_p.tile([128, C], bf16)
                    nc.gpsimd.dma_start(
                        out=xb, in_=x_ap[b, 128 * tt : 128 * (tt + 1), :]
                    )
                    xbfs.append(xb)

                # --- x -> x^T (bf16) via PE transpose ---
                xts = [
                    xt_p.tile([128, T], bf16, name=f"xt{cc}", tag="xt")
                    for cc in range(8)
                ]
                for tt8 in range(2):
                    for cc in range(8):
                        ps = trans_ps.tile([128, 1024], bf16)
                        for m in range(8):
                            nc.tensor.transpose(
                                ps[:, 128 * m : 128 * (m + 1)],
                                xbfs[8 * tt8 + m][:, 128 * cc : 128 * (cc + 1)],
                                ident,
                            )
                        nc.vector.tensor_copy(
                            out=xts[cc][:, 1024 * tt8 : 1024 * (tt8 + 1)], in_=ps
                        )

                # --- projections ---
                qT = qk_p.tile([128, T], bf16)
                kT = qk_p.tile([128, T], bf16)
                for wt, dst in ((WT["q"], qT), (WT["k"], kT)):
                    for tt4 in range(4):
                        ps = mm_ps.tile([128, 512], f32)
                        for cc in range(8):
                            nc.tensor.matmul(
                                ps,
                                wt[:, 128 * cc : 128 * (cc + 1)],
                                xts[cc][:, 512 * tt4 : 512 * (tt4 + 1)],
                                start=(cc == 0),
                                stop=(cc == 7),
                            )
                        nc.scalar.copy(
                            out=dst[:, 512 * tt4 : 512 * (tt4 + 1)], in_=ps
                        )
                # v^T = Wv @ x^T (like q/k), then PE-transpose back to natural
                # [s, h] blocks for the P.V contraction.
                vT = qk_p.tile([128, T], bf16)
                vas = []
                for tt4 in range(4):
                    ps = mm_ps.tile([128, 512], f32)
                    for cc in range(8):
                        nc.tensor.matmul(
                            ps,
                            WT["v"][:, 128 * cc : 128 * (cc + 1)],
                            xts[cc][:, 512 * tt4 : 512 * (tt4 + 1)],
                            start=(cc == 0),
                            stop=(cc == 7),
                        )
                    nc.scalar.copy(out=vT[:, 512 * tt4 : 512 * (tt4 + 1)], in_=ps)
                    # transpose this slice's 4 natural [s, h] blocks right
                    # away: PE works while the ACT copy chain drains, and
                    # vas[0..3] are ready earlier for P.V row 0
                    for a in range(4):
                        ss = 4 * tt4 + a
                        psv = trans_ps.tile([128, 512], bf16, tag="ps")
                        nc.tensor.transpose(
                            psv[:, 0:128], vT[:, 128 * ss : 128 * (ss + 1)], ident
                        )
                        va = va_p.tile([128, H + 1], bf16, tag="va", name="va")
                        nc.vector.tensor_copy(out=va[:, 0:128], in_=psv[:, 0:128])
                        nc.gpsimd.memset(va[:, 128:129], 1.0)
                        vas.append(va)

                # --- scores (transposed), exp, and P.V interleaved ---
                out_sb = osb_p.tile([128, 16 * H], f32)
                prows = []
                for ss in range(16):
                    pr = pr_p.tile([128, T], bf16)
                    prows.append(pr)
                    for tq in range(ss // 4, 4):
                        c0 = 512 * tq
                        x0 = max(128 * ss, c0)  # first causal-needed column
                        d0 = x0 - c0
                        sh = srow_ps.tile([128, 512], f32)
                        nc.tensor.matmul(
                            sh[:, d0:512],
                            kT[:, 128 * ss : 128 * (ss + 1)],
                            qT[:, x0 : c0 + 512],
                            start=True,
                            stop=True,
                        )
                        nc.scalar.activation(
                            out=pr[:, x0 : c0 + 512],
                            in_=sh[:, d0:512],
                            func=Exp,
                            scale=SC,
                        )
                    nc.vector.tensor_mul(
                        pr[:, 128 * ss : 128 * (ss + 1)],
                        pr[:, 128 * ss : 128 * (ss + 1)],
                        trimask,
                    )
                    pv = pv_ps.tile([128, H + 1], f32)
                    for j in range(ss + 1):
                        nc.tensor.matmul(
                            pv,
                            prows[j][:, 128 * ss : 128 * (ss + 1)],
                            vas[j],
                            start=(j == 0),
                            stop=(j == ss),
                        )
                    rc = rc_p.tile([128, 1], f32)
                    nc.vector.reciprocal(rc, pv[:, 128:129])
                    nc.vector.tensor_mul(
                        out_sb[:, H * ss : H * (ss + 1)],
                        pv[:, 0:128],
                        rc.broadcast_to([128, H]),
                    )
                # out_sb[p, (g h)] -> out[b, 128g+p, h]; split DMAs so the
                # final transfer after the last normalize is small
                np_split = 4 if b == BL - 1 else 2
                npc = 2048 // np_split
                for hh in range(np_split):
                    nc.sync.dma_start(
                        out=out_ap[b, npc * hh : npc * (hh + 1), :].rearrange(
                            "(g p) h -> p g h", p=128
                        ),
                        in_=out_sb[
                            :, npc // 128 * H * hh : npc // 128 * H * (hh + 1)
                        ].rearrange("p (g h) -> p g h", h=H),
                    )

    nc.compile()
    return nc


def _get_nc():
    if "nc" not in _CACHE:
        _CACHE["nc"] = _build()
    return _CACHE["nc"]


def kernel(x, Wk, Wq, Wv, _trace=False):
    from concourse.bass_utils import run_bass_kernel_spmd

    x = np.ascontiguousarray(np.asarray(x, dtype=np.float32))
    Wk = np.ascontiguousarray(np.asarray(Wk, dtype=np.float32))
    Wq = np.ascontiguousarray(np.asarray(Wq, dtype=np.float32))
    Wv = np.ascontiguousarray(np.asarray(Wv, dtype=np.float32))
    assert x.shape == (B, T, C)

    nc = _get_nc()
    in_maps = [
        {"x": x[i * BL : (i + 1) * BL], "Wk": Wk, "Wq": Wq, "Wv": Wv}
        for i in range(NCORES)
    ]
    res = run_bass_kernel_spmd(nc, in_maps, list(range(NCORES)), trace=_trace)
    out = np.concatenate([res.results[i]["out"] for i in range(NCORES)], axis=0)
    if _trace:
        _CACHE["last_results"] = res
    return out

